# revision 1
# baseline (speedup 1.0000x reference)
"""DenseGATConv Bass/Tile kernel for Trainium2, SPMD over 8 NeuronCores.

Problem (B=4, N=2048, F=128, H=4, C=64):
  xh = (x @ W).reshape(B,N,H,C)
  a_src[b,j,h] = xh . att_src ; a_dst[b,i,h] = xh . att_dst
  s = a_src[j] + a_dst[i];  alpha = softmax_j(mask(adj+I, leaky_relu(s, 0.2)))
  out[b,i] = concat_h(sum_j alpha * xh[b,j,h,:]) + bias

Key algebraic transform (no exp over the N*N*H grid, no explicit softmax
normalizer subtraction):
  exp(lrelu(s)) = exp(a_src_j) * exp(a_dst_i) * max(Q'_i R'_j, 1),
      Q' = exp(-0.8 a_dst), R' = exp(-0.8 a_src)
  exp(a_dst_i) cancels in the softmax ratio. Fold exp(a_src_j) into the
  max: the masked grid weight becomes
      G[j,i] = adjT[j,i] * max(Q'_i * exp(0.2 a_src_j), exp(a_src_j))
  which is exactly 2 DVE ops per (j-tile, head):
      T = tensor_scalar(Q'_bcast, s1=exp(.2 a_src), s2=exp(a_src); mult, max)
      G = tensor_tensor(T, adjT)           # fused over all 4 heads
  Then PE accumulates num/den with one stationary load per (tile, head):
      acc[h][c,i] += xh1[j, c|1]^T @ G[j, h*ID + i]      (fp16, f32 PSUM)
  row 64 of acc is the softmax denominator (ones column in xh1).
  Epilogue: PSUM->SBUF, PE-transpose [65,128] blocks, divide by den and
  add bias with DVE, DMA out f32 rows.

Sharding: core = b*2 + ihalf; each core owns 1024 destination rows of one
batch and reads that batch's full source side (adj slice pre-transposed,
self-loops added, fp16-cast on host; weights pre-folded with the per-head
attention vectors, all exp argument scalings baked into extra projection
columns).
"""

import numpy as np

import concourse.bacc as bacc
import concourse.bass as bass
import concourse.tile as tile
from concourse import mybir
from concourse.bass_utils import run_bass_kernel_spmd
from concourse.masks import make_identity

B, N, F = 4, 2048, 128
H, C = 4, 64
HC = H * C
NEG_SLOPE = 0.2
import os
TBUFS = int(os.environ.get('TBUFS', 4))
GBUFS = int(os.environ.get('GBUFS', 5))
ABUFS = int(os.environ.get('ABUFS', 3))
N_CORES = 8
ID = N // 2          # dest rows per core
NT = N // 128        # 16 source tiles
NKD = ID // 512      # 2 dest 512-chunks
F32 = mybir.dt.float32
F16 = mybir.dt.float16

_NC_CACHE = {}


def build_nc(reps: int = 1):
    nc = bacc.Bacc("TRN2", target_bir_lowering=False, debug=False, num_devices=1)

    d_xT = nc.dram_tensor("xT", [F, N], F32, kind="ExternalInput").ap()
    d_xTd = nc.dram_tensor("xTd", [F, ID], F32, kind="ExternalInput").ap()
    d_adjT = nc.dram_tensor("adjT", [NT, 128, ID], F16, kind="ExternalInput").ap()
    d_wcat = nc.dram_tensor("Wcat", [F, HC + 8], F32, kind="ExternalInput").ap()
    d_wadst = nc.dram_tensor("Wadst", [F, H], F32, kind="ExternalInput").ap()
    d_bias = nc.dram_tensor("biasv", [1, HC], F32, kind="ExternalInput").ap()
    d_out = nc.dram_tensor("out", [ID, HC], F32, kind="ExternalOutput").ap()

    EXP = mybir.ActivationFunctionType.Exp
    CPY = mybir.ActivationFunctionType.Copy

    with tile.TileContext(nc) as tc:
        with tc.tile_pool(name="const", bufs=1) as const:
            ident = const.tile([128, 128], F32)
            make_identity(nc, ident)
            ones1 = const.tile([1, 128], F32)
            nc.vector.memset(ones1, 1.0)

            # preload the exp table set while input DMAs run
            scratch1 = const.tile([1, 4], F32)
            nc.scalar.activation(scratch1, ones1[0:1, 0:4], EXP)

            wcat = const.tile([F, HC + 8], F32)
            nc.sync.dma_start(out=wcat, in_=d_wcat)
            wadst = const.tile([F, H], F32)
            nc.sync.dma_start(out=wadst, in_=d_wadst)
            xTd = const.tile([F, ID], F32)
            for c in range(2):
                nc.sync.dma_start(out=xTd[:, c * 512:(c + 1) * 512],
                                  in_=d_xTd[:, c * 512:(c + 1) * 512])
            xT = const.tile([F, N], F32)
            for c in range(4):
                nc.sync.dma_start(out=xT[:, c * 512:(c + 1) * 512],
                                  in_=d_xT[:, c * 512:(c + 1) * 512])
            bias_sb = const.tile([1, HC], F32)
            nc.sync.dma_start(out=bias_sb, in_=d_bias)

            # persistent per-core tensors
            xh1 = const.tile([128, NT, H, 65], F16)     # [xh | 1] per (t,h)
            expv = const.tile([128, NT, 8], F32)        # exp(.2 a_src) | exp(a_src)
            q_bc = const.tile([128, H, ID], F16)        # Q' broadcast per head
            bias_bc = const.tile([128, HC], F32)

            # ---------------- phase A: projections ----------------
            with tc.tile_pool(name="psA", bufs=2, space="PSUM") as psA, \
                 tc.tile_pool(name="psD", bufs=3, space="PSUM") as psDp, \
                 tc.tile_pool(name="psB", bufs=2, space="PSUM") as psBp:
                # ones column of every xh1 block (cols 0:64 written below)
                nc.gpsimd.memset(xh1[:, :, :, 64:65], 1.0)
                sc_a = nc.enter_named_scope("phA", False)
                # --- q_bc prefix first: it gates the grid loop. ACT does the
                # small exps, DVE does the psum->sbuf broadcast copies so the
                # prefix finishes fast; ACT's per-tile work below then overlaps
                # the grid.
                qrow = const.tile([1, H, ID], F16)
                for h in range(H):
                    for k in range(NKD):
                        psd = psDp.tile([1, 512], F32)
                        nc.tensor.matmul(psd, wadst[:, h:h + 1],
                                         xTd[:, k * 512:(k + 1) * 512],
                                         start=True, stop=True)
                        nc.scalar.activation(
                            qrow[0:1, h, k * 512:(k + 1) * 512], psd, EXP)
                # broadcast Q' rows to all 128 partitions via a DRAM bounce,
                # one head at a time so the grid's first tiles start sooner
                with tc.tile_pool(name="dscr", bufs=1, space="DRAM") as dscr:
                    qscr = dscr.tile([H, ID], F16)
                    for h in range(H):
                        nc.gpsimd.dma_start(out=qscr[h:h + 1, :],
                                            in_=qrow[0:1, h, :])
                        hrow = qscr[h:h + 1, :]
                        qscr_bcast = bass.AP(
                            tensor=hrow.tensor, offset=hrow.offset,
                            ap=[[0, 128]] + list(hrow.ap[1:]))
                        nc.gpsimd.dma_start(out=q_bc[:, h, :], in_=qscr_bcast)
                # projection tiles; grid tile t can start once tile t is done
                for t in range(NT):
                    ps = psA.tile([128, HC + 8], F32)
                    nc.tensor.matmul(ps, xT[:, t * 128:(t + 1) * 128], wcat,
                                     start=True, stop=True)
                    # exp of the 8 pre-scaled projection cols
                    nc.scalar.activation(expv[:, t, :], ps[:, HC:HC + 8], EXP)
                    # raw xh into the 65-column head blocks
                    nc.scalar.activation(xh1[:, t, :, 0:64], ps[:, 0:HC], CPY)
                # bias broadcast (only needed by the epilogue)
                psb2 = psBp.tile([128, HC], F32, tag="psbias", bufs=1)
                nc.tensor.matmul(psb2, ones1, bias_sb, start=True, stop=True)
                nc.scalar.activation(bias_bc, psb2, CPY)
                nc.leave_named_scope("phA", sc_a[0], False)

            # ---------------- phase B: grid + matmul accumulate ----------------
            with tc.tile_pool(name="ep_sb", bufs=1) as epsb:
                with tc.tile_pool(name="acc", bufs=1, space="PSUM") as accp:
                    acc = {}
                    for h in range(H):
                        acc_t = accp.tile([65, ID], F32, tag=f"acc{h}",
                                          name=f"acc{h}")
                        acc[h] = acc_t

                    sc_b = nc.enter_named_scope("phB", False)
                    with tc.tile_pool(name="adj", bufs=ABUFS) as adjp, \
                         tc.tile_pool(name="grid", bufs=4) as gridp:
                        for rep in range(reps):
                            for t in range(NT):
                                adjt = adjp.tile([128, ID], F16)
                                nc.sync.dma_start(out=adjt, in_=d_adjT[t])
                                t_all = gridp.tile([128, H, ID], F16, tag="T", bufs=TBUFS)
                                for h in range(H):
                                    # T2 = max(Q'_i * exp(.2 a_src_j), exp(a_src_j))
                                    nc.vector.tensor_scalar(
                                        out=t_all[:, h, :], in0=q_bc[:, h, :],
                                        scalar1=expv[:, t, h:h + 1],
                                        scalar2=expv[:, t, 4 + h:5 + h],
                                        op0=mybir.AluOpType.mult,
                                        op1=mybir.AluOpType.max)
                                g = gridp.tile([128, H, ID], F16, tag="G", bufs=GBUFS)
                                adj_rep2 = bass.AP(
                                    tensor=adjt.tensor, offset=adjt.offset,
                                    ap=[adjt.ap[0], [0, 2]] + list(adjt.ap[1:]))
                                for hp in range(2):
                                    nc.vector.tensor_tensor(
                                        out=g[:, hp * 2:(hp + 1) * 2, :],
                                        in0=t_all[:, hp * 2:(hp + 1) * 2, :],
                                        in1=adj_rep2,
                                        op=mybir.AluOpType.mult)
                                first = (rep == 0 and t == 0)
                                last = (rep == reps - 1 and t == NT - 1)
                                for h in range(H):
                                    for k in range(NKD):
                                        nc.tensor.matmul(
                                            acc[h][:, k * 512:(k + 1) * 512],
                                            xh1[:, t, h, :],
                                            g[:, h, k * 512:(k + 1) * 512],
                                            start=first, stop=last)

                    nc.leave_named_scope("phB", sc_b[0], False)
                    sc_c = nc.enter_named_scope("phC", False)
                    # evacuate accumulators to SBUF (ACT is close to PSUM)
                    s_tiles = {}
                    for h in range(H):
                        s = epsb.tile([65, ID], F32, tag=f"s{h}", name=f"s{h}")
                        if h % 2 == 0:
                            nc.scalar.activation(s, acc[h], CPY)
                        else:
                            nc.vector.tensor_copy(s, acc[h])
                        for k in range(NKD):
                            s_tiles[(h, k)] = s[:, k * 512:(k + 1) * 512]

                # acc PSUM released here
                # ------------- phase C: transpose + divide + bias + out -------------
                with tc.tile_pool(name="ep_ps", bufs=8, space="PSUM") as epps, \
                     tc.tile_pool(name="ep_sm", bufs=4) as epsm, \
                     tc.tile_pool(name="outp", bufs=2) as outp:
                    for k in range(NKD):
                        osb = outp.tile([128, 4, HC], F32, tag="osb", name="osb")
                        for h in range(H):
                            pt = epps.tile([128, 4, 65], F32)
                            for kk in range(4):
                                nc.tensor.transpose(
                                    pt[:, kk, :],
                                    s_tiles[(h, k)][:, kk * 128:(kk + 1) * 128],
                                    ident[0:65, 0:65])
                            rec = epsm.tile([128, 4, 1], F32)
                            nc.vector.reciprocal(rec, pt[:, :, 64:65])
                            rec_rep = bass.AP(
                                tensor=rec.tensor, offset=rec.offset,
                                ap=[rec.ap[0], rec.ap[1], [0, 64]])
                            bias_rep = bass.AP(
                                tensor=bias_bc.tensor,
                                offset=bias_bc.offset + h * 64,
                                ap=[bias_bc.ap[0], [0, 4], [1, 64]])
                            nc.vector.tensor_tensor(
                                out=osb[:, :, h * 64:(h + 1) * 64],
                                in0=pt[:, :, 0:64], in1=rec_rep,
                                op=mybir.AluOpType.mult)
                            nc.vector.tensor_tensor(
                                out=osb[:, :, h * 64:(h + 1) * 64],
                                in0=osb[:, :, h * 64:(h + 1) * 64], in1=bias_rep,
                                op=mybir.AluOpType.add)
                        # destination rows are host-permuted so partition p
                        # holds 4 consecutive output rows: one contiguous 4KB
                        # descriptor per partition instead of four 1KB ones
                        blk = d_out[k * 512:(k + 1) * 512, :]
                        out_ap = bass.AP(
                            tensor=blk.tensor, offset=blk.offset,
                            ap=[[4 * HC, 128], [HC, 4], [1, HC]])
                        nc.sync.dma_start(out=out_ap, in_=osb)
                    nc.leave_named_scope("phC", sc_c[0], False)

    nc.compile()
    return nc


def _get_nc(reps: int = 1):
    if reps not in _NC_CACHE:
        _NC_CACHE[reps] = build_nc(reps)
    return _NC_CACHE[reps]


def make_in_maps(x, adj, W, att_src, att_dst, bias):
    x = np.asarray(x, dtype=np.float32)
    adj = np.asarray(adj, dtype=np.float32)
    W = np.asarray(W, dtype=np.float32)
    att_src = np.asarray(att_src, dtype=np.float32)
    att_dst = np.asarray(att_dst, dtype=np.float32)
    bias = np.asarray(bias, dtype=np.float32)

    # weight prep: fold per-head attention dots into projection columns
    wa_src = np.stack([W[:, h * C:(h + 1) * C] @ att_src[h] for h in range(H)], 1)
    wa_dst = np.stack([W[:, h * C:(h + 1) * C] @ att_dst[h] for h in range(H)], 1)
    wcat = np.concatenate([W, 0.2 * wa_src, wa_src], axis=1)
    wcat = np.ascontiguousarray(wcat, dtype=np.float32)          # [F, 264]
    wadst = np.ascontiguousarray(-0.8 * wa_dst, dtype=np.float32)  # [F, 4]

    adjl = adj.copy()
    idx = np.arange(N)
    adjl[:, idx, idx] = 1.0

    # destination-row permutation: kernel position i' = kk*128 + p within each
    # 512-block maps to original row p*4 + kk, so the output DMA writes 4KB
    # contiguous chunks per partition
    perm = np.concatenate([kb * 512 + (np.arange(512) % 128) * 4 + np.arange(512) // 128
                           for kb in range(ID // 512)])

    in_maps = []
    for c in range(N_CORES):
        b, half = c // 2, c % 2
        xT = np.ascontiguousarray(x[b].T, dtype=np.float32)
        xTd = np.ascontiguousarray(x[b, half * ID:(half + 1) * ID, :].T[:, perm],
                                   dtype=np.float32)
        adjT = np.ascontiguousarray(
            adjl[b].T[:, half * ID:(half + 1) * ID][:, perm]).astype(np.float16)
        in_maps.append({
            "xT": xT,
            "xTd": xTd,
            "adjT": adjT.reshape(NT, 128, ID),
            "Wcat": wcat,
            "Wadst": wadst,
            "biasv": bias.reshape(1, HC),
        })
    return in_maps


def assemble(results):
    out = np.empty((B, N, HC), dtype=np.float32)
    for c in range(N_CORES):
        b, half = c // 2, c % 2
        out[b, half * ID:(half + 1) * ID, :] = results[c]["out"]
    return out


def kernel(x, adj, W, att_src, att_dst, bias):
    nc = _get_nc(1)
    in_maps = make_in_maps(x, adj, W, att_src, att_dst, bias)
    res = run_bass_kernel_spmd(nc, in_maps, list(range(N_CORES)))
    return assemble(res.results)



# revision 4
# speedup vs baseline: 1.0356x; 1.0356x over previous
"""DenseGATConv Bass/Tile kernel for Trainium2, SPMD over 8 NeuronCores.

Problem (B=4, N=2048, F=128, H=4, C=64):
  xh = (x @ W).reshape(B,N,H,C)
  a_src[b,j,h] = xh . att_src ; a_dst[b,i,h] = xh . att_dst
  s = a_src[j] + a_dst[i];  alpha = softmax_j(mask(adj+I, leaky_relu(s, 0.2)))
  out[b,i] = concat_h(sum_j alpha * xh[b,j,h,:]) + bias

Key algebraic transform (no exp over the N*N*H grid, no explicit softmax
normalizer subtraction):
  exp(lrelu(s)) / exp(a_dst_i) = max(E2_j, E1_j * Q'_i),
      E1 = exp(0.2 a_src), E2 = exp(a_src), Q' = exp(-0.8 a_dst)
  The masked grid weight is
      G[j,i] = adjT[j,i] * max(Q'_i * E1_j, E2_j)
  computed as 2 DVE ops per (j-tile, head):
      T = tensor_scalar(Q'_bcast, s1=E1, s2=E2; mult, max)
      G = tensor_tensor(T, adjT)           # 2 heads per op via rep-AP
  Then PE accumulates num/den with one stationary load per (tile, head):
      acc[h][c,i] += xh1[j, c|1]^T @ G[j, h*ID + i]      (fp16, f32 PSUM)
  row 64 of acc is the softmax denominator (ones column in xh1).

  Q'_i is broadcast to all 128 partitions with a K=1 ones-stationary
  matmul (PSUM bounce) instead of a DRAM roundtrip, so the grid pipeline
  starts ~8us earlier.  Epilogue: PSUM->SBUF evac, PE-transpose into a
  single padded PSUM tile per 512-chunk, then one batched reciprocal /
  multiply / bias-add per chunk and a contiguous-per-partition DMA out.

Sharding: core = b*2 + ihalf; each core owns 1024 destination rows of one
batch and reads that batch's full source side (adj slice pre-transposed,
self-loops added, fp16-cast on host; weights pre-folded with the per-head
attention vectors, exp argument scalings baked into extra projection
columns; x / W / projections run in fp16).
"""

import numpy as np

import concourse.bacc as bacc
import concourse.bass as bass
import concourse.tile as tile
from concourse import mybir
from concourse.bass_utils import run_bass_kernel_spmd
from concourse.masks import make_identity

B, N, F = 4, 2048, 128
H, C = 4, 64
HC = H * C
NEG_SLOPE = 0.2
import os
TBUFS = int(os.environ.get('TBUFS', 4))
GBUFS = int(os.environ.get('GBUFS', 5))
ABUFS = int(os.environ.get('ABUFS', 3))
N_CORES = 8
ID = N // 2          # dest rows per core
NT = N // 128        # 16 source tiles
NKD = ID // 512      # 2 dest 512-chunks
F32 = mybir.dt.float32
F16 = mybir.dt.float16

_NC_CACHE = {}


def build_nc(reps: int = 1):
    nc = bacc.Bacc("TRN2", target_bir_lowering=False, debug=False, num_devices=1)

    d_xT = nc.dram_tensor("xT", [F, N], F16, kind="ExternalInput").ap()
    d_xTd = nc.dram_tensor("xTd", [F, ID], F16, kind="ExternalInput").ap()
    d_adjT = nc.dram_tensor("adjT", [NT, 128, ID], F16, kind="ExternalInput").ap()
    d_wcat = nc.dram_tensor("Wcat", [F, HC + 8], F16, kind="ExternalInput").ap()
    d_wadst = nc.dram_tensor("Wadst", [F, H], F16, kind="ExternalInput").ap()
    d_bias = nc.dram_tensor("biasv", [1, HC], F32, kind="ExternalInput").ap()
    d_out = nc.dram_tensor("out", [ID, HC], F32, kind="ExternalOutput").ap()

    EXP = mybir.ActivationFunctionType.Exp
    CPY = mybir.ActivationFunctionType.Copy

    with tile.TileContext(nc) as tc:
        with tc.tile_pool(name="const", bufs=1) as const:
            ident = const.tile([128, 128], F32)
            make_identity(nc, ident)
            ones1 = const.tile([1, 128], F32)
            nc.vector.memset(ones1, 1.0)
            ones16 = const.tile([1, 128], F16)
            nc.vector.memset(ones16, 1.0)

            # preload the exp table set while input DMAs run
            scratch1 = const.tile([1, 4], F32)
            nc.scalar.activation(scratch1, ones1[0:1, 0:4], EXP)

            wcat = const.tile([F, HC + 8], F16)
            nc.sync.dma_start(out=wcat, in_=d_wcat)
            wadst = const.tile([F, H], F16)
            nc.sync.dma_start(out=wadst, in_=d_wadst)
            xTd = const.tile([F, ID], F16)
            nc.sync.dma_start(out=xTd, in_=d_xTd)
            xT = const.tile([F, N], F16)
            for c in range(2):
                nc.sync.dma_start(out=xT[:, c * 1024:(c + 1) * 1024],
                                  in_=d_xT[:, c * 1024:(c + 1) * 1024])
            bias_sb = const.tile([1, HC], F32)
            nc.sync.dma_start(out=bias_sb, in_=d_bias)

            # persistent per-core tensors
            xh1 = const.tile([128, NT, H, 65], F16)     # [xh | 1] per (t,h)
            expv = const.tile([128, NT, 8], F32)        # exp(.2 a_src) | exp(a_src)
            q_bc = const.tile([128, H, ID], F16)        # Q' broadcast per head
            qrow = const.tile([1, H, ID], F16)          # exp(-0.8 a_dst), row form
            bias_bc = const.tile([128, HC], F32)

            # ---------------- phase A: projections ----------------
            with tc.tile_pool(name="psA", bufs=2, space="PSUM") as psA, \
                 tc.tile_pool(name="psD", bufs=2, space="PSUM") as psDp, \
                 tc.tile_pool(name="psQ", bufs=2, space="PSUM") as psQp, \
                 tc.tile_pool(name="psB", bufs=2, space="PSUM") as psBp:
                # ones column of every xh1 block (cols 0:64 written below)
                nc.gpsimd.memset(xh1[:, :, :, 64:65], 1.0)
                sc_a = nc.enter_named_scope("phA", False)
                # --- q_bc prefix first: it gates the grid loop.  a_dst
                # projection -> exp -> K=1 ones-stationary matmul broadcasts
                # each Q' row to all 128 partitions via PSUM (no DRAM bounce).
                for h in range(H):
                    for k in range(NKD):
                        psd = psDp.tile([1, 512], F32)
                        nc.tensor.matmul(psd, wadst[:, h:h + 1],
                                         xTd[:, k * 512:(k + 1) * 512],
                                         start=True, stop=True)
                        nc.scalar.activation(qrow[0:1, h, k * 512:(k + 1) * 512],
                                             psd, EXP)
                        psq = psQp.tile([128, 512], F32)
                        nc.tensor.matmul(psq, ones16,
                                         qrow[0:1, h, k * 512:(k + 1) * 512],
                                         start=True, stop=True)
                        nc.vector.tensor_copy(
                            q_bc[:, h, k * 512:(k + 1) * 512], psq)
                # projection tiles; grid tile t can start once tile t is done
                for t in range(NT):
                    ps = psA.tile([128, HC + 8], F32)
                    nc.tensor.matmul(ps, xT[:, t * 128:(t + 1) * 128], wcat,
                                     start=True, stop=True)
                    # exp of the 8 pre-scaled projection cols
                    nc.scalar.activation(expv[:, t, :], ps[:, HC:HC + 8], EXP)
                    # raw xh into the 65-column head blocks
                    nc.scalar.activation(xh1[:, t, :, 0:64], ps[:, 0:HC], CPY)
                # bias broadcast (only needed by the epilogue)
                psb2 = psBp.tile([128, HC], F32, tag="psbias", bufs=1)
                nc.tensor.matmul(psb2, ones1, bias_sb, start=True, stop=True)
                nc.scalar.activation(bias_bc, psb2, CPY)
                nc.leave_named_scope("phA", sc_a[0], False)

            # ---------------- phase B: grid + matmul accumulate ----------------
            with tc.tile_pool(name="ep_sb", bufs=1) as epsb:
                with tc.tile_pool(name="acc", bufs=1, space="PSUM") as accp:
                    acc = {}
                    for h in range(H):
                        acc_t = accp.tile([65, ID], F32, tag=f"acc{h}",
                                          name=f"acc{h}")
                        acc[h] = acc_t

                    sc_b = nc.enter_named_scope("phB", False)
                    with tc.tile_pool(name="adj", bufs=ABUFS) as adjp, \
                         tc.tile_pool(name="grid", bufs=4) as gridp:
                        for rep in range(reps):
                            for t in range(NT):
                                adjt = adjp.tile([128, ID], F16)
                                nc.sync.dma_start(out=adjt, in_=d_adjT[t])
                                t_all = gridp.tile([128, H, ID], F16, tag="T", bufs=TBUFS)
                                for h in range(H):
                                    # T2 = max(Q'_i * exp(.2 a_src_j), exp(a_src_j))
                                    nc.vector.tensor_scalar(
                                        out=t_all[:, h, :], in0=q_bc[:, h, :],
                                        scalar1=expv[:, t, h:h + 1],
                                        scalar2=expv[:, t, 4 + h:5 + h],
                                        op0=mybir.AluOpType.mult,
                                        op1=mybir.AluOpType.max)
                                g = gridp.tile([128, H, ID], F16, tag="G", bufs=GBUFS)
                                adj_rep2 = bass.AP(
                                    tensor=adjt.tensor, offset=adjt.offset,
                                    ap=[adjt.ap[0], [0, 2]] + list(adjt.ap[1:]))
                                for hp in range(2):
                                    nc.vector.tensor_tensor(
                                        out=g[:, hp * 2:(hp + 1) * 2, :],
                                        in0=t_all[:, hp * 2:(hp + 1) * 2, :],
                                        in1=adj_rep2,
                                        op=mybir.AluOpType.mult)
                                first = (rep == 0 and t == 0)
                                last = (rep == reps - 1 and t == NT - 1)
                                for h in range(H):
                                    for k in range(NKD):
                                        nc.tensor.matmul(
                                            acc[h][:, k * 512:(k + 1) * 512],
                                            xh1[:, t, h, :],
                                            g[:, h, k * 512:(k + 1) * 512],
                                            start=first, stop=last)

                    nc.leave_named_scope("phB", sc_b[0], False)
                    sc_c = nc.enter_named_scope("phC", False)
                    # evacuate accumulators to SBUF (ACT is close to PSUM)
                    s_tiles = {}
                    for h in range(H):
                        s = epsb.tile([65, ID], F32, tag=f"s{h}", name=f"s{h}")
                        if h % 2 == 0:
                            nc.scalar.activation(s, acc[h], CPY)
                        else:
                            nc.vector.tensor_copy(s, acc[h])
                        s_tiles[h] = s

                # acc PSUM released here
                # ------------- phase C: transpose + divide + bias + out -------------
                # One padded PSUM tile per 512-chunk holds all 16 transposed
                # [65,128] blocks (h, kk); divide/bias run as 2 batched DVE ops.
                with tc.tile_pool(name="ep_ps", bufs=2, space="PSUM") as epps, \
                     tc.tile_pool(name="ep_sm", bufs=2) as epsm, \
                     tc.tile_pool(name="outp", bufs=2) as outp:
                    for k in range(NKD):
                        pt = epps.tile([128, H, 4, 128], F32)
                        for h in range(H):
                            for kk in range(4):
                                nc.tensor.transpose(
                                    pt[:, h, kk, 0:65],
                                    s_tiles[h][:, k * 512 + kk * 128:
                                               k * 512 + (kk + 1) * 128],
                                    ident[0:65, 0:65])
                        rec = epsm.tile([128, H, 4, 1], F32)
                        nc.vector.reciprocal(rec, pt[:, :, :, 64:65])
                        osb = outp.tile([128, 4, HC], F32, tag="osb", name="osb")
                        # osb viewed as [part, h, kk, c] for the batched ops
                        osb_v = bass.AP(
                            tensor=osb.tensor, offset=osb.offset,
                            ap=[osb.ap[0], [64, H], [HC, 4], [1, 64]])
                        rec_rep = bass.AP(
                            tensor=rec.tensor, offset=rec.offset,
                            ap=[rec.ap[0], [4, H], [1, 4], [0, 64]])
                        bias_rep = bass.AP(
                            tensor=bias_bc.tensor, offset=bias_bc.offset,
                            ap=[bias_bc.ap[0], [64, H], [0, 4], [1, 64]])
                        nc.vector.tensor_tensor(
                            out=osb_v, in0=pt[:, :, :, 0:64], in1=rec_rep,
                            op=mybir.AluOpType.mult)
                        nc.vector.tensor_tensor(
                            out=osb_v, in0=osb_v, in1=bias_rep,
                            op=mybir.AluOpType.add)
                        # destination rows are host-permuted so partition p
                        # holds 4 consecutive output rows: one contiguous 4KB
                        # descriptor per partition instead of four 1KB ones
                        blk = d_out[k * 512:(k + 1) * 512, :]
                        out_ap = bass.AP(
                            tensor=blk.tensor, offset=blk.offset,
                            ap=[[4 * HC, 128], [HC, 4], [1, HC]])
                        nc.sync.dma_start(out=out_ap, in_=osb)
                    nc.leave_named_scope("phC", sc_c[0], False)

    nc.compile()
    return nc


def _get_nc(reps: int = 1):
    if reps not in _NC_CACHE:
        _NC_CACHE[reps] = build_nc(reps)
    return _NC_CACHE[reps]


def make_in_maps(x, adj, W, att_src, att_dst, bias):
    x = np.asarray(x, dtype=np.float32)
    adj = np.asarray(adj, dtype=np.float32)
    W = np.asarray(W, dtype=np.float32)
    att_src = np.asarray(att_src, dtype=np.float32)
    att_dst = np.asarray(att_dst, dtype=np.float32)
    bias = np.asarray(bias, dtype=np.float32)

    # weight prep: fold per-head attention dots into projection columns
    wa_src = np.stack([W[:, h * C:(h + 1) * C] @ att_src[h] for h in range(H)], 1)
    wa_dst = np.stack([W[:, h * C:(h + 1) * C] @ att_dst[h] for h in range(H)], 1)
    wcat = np.concatenate([W, 0.2 * wa_src, wa_src], axis=1)
    wcat = np.ascontiguousarray(wcat, dtype=np.float16)          # [F, 264]
    wadst = np.ascontiguousarray(-0.8 * wa_dst, dtype=np.float16)  # [F, 4]

    adjl = adj.copy()
    idx = np.arange(N)
    adjl[:, idx, idx] = 1.0

    # destination-row permutation: kernel position i' = kk*128 + p within each
    # 512-block maps to original row p*4 + kk, so the output DMA writes 4KB
    # contiguous chunks per partition
    perm = np.concatenate([kb * 512 + (np.arange(512) % 128) * 4 + np.arange(512) // 128
                           for kb in range(ID // 512)])

    in_maps = []
    for c in range(N_CORES):
        b, half = c // 2, c % 2
        xT = np.ascontiguousarray(x[b].T, dtype=np.float16)
        xTd = np.ascontiguousarray(x[b, half * ID:(half + 1) * ID, :].T[:, perm],
                                   dtype=np.float16)
        adjT = np.ascontiguousarray(
            adjl[b].T[:, half * ID:(half + 1) * ID][:, perm]).astype(np.float16)
        in_maps.append({
            "xT": xT,
            "xTd": xTd,
            "adjT": adjT.reshape(NT, 128, ID),
            "Wcat": wcat,
            "Wadst": wadst,
            "biasv": bias.reshape(1, HC),
        })
    return in_maps


def assemble(results):
    out = np.empty((B, N, HC), dtype=np.float32)
    for c in range(N_CORES):
        b, half = c // 2, c % 2
        out[b, half * ID:(half + 1) * ID, :] = results[c]["out"]
    return out


def kernel(x, adj, W, att_src, att_dst, bias):
    nc = _get_nc(1)
    in_maps = make_in_maps(x, adj, W, att_src, att_dst, bias)
    res = run_bass_kernel_spmd(nc, in_maps, list(range(N_CORES)))
    return assemble(res.results)


# revision 8
# speedup vs baseline: 1.0395x; 1.0037x over previous
"""DenseGATConv Bass/Tile kernel for Trainium2, SPMD over 8 NeuronCores.

Problem (B=4, N=2048, F=128, H=4, C=64):
  xh = (x @ W).reshape(B,N,H,C)
  a_src[b,j,h] = xh . att_src ; a_dst[b,i,h] = xh . att_dst
  s = a_src[j] + a_dst[i];  alpha = softmax_j(mask(adj+I, leaky_relu(s, 0.2)))
  out[b,i] = concat_h(sum_j alpha * xh[b,j,h,:]) + bias

Algebra (no exp over the N*N*H grid, no softmax normalizer subtraction):
  exp(lrelu(s)) / exp(a_dst_i) = max(E1_j * Q'_i, E2_j),
      E1 = exp(0.2 a_src), E2 = exp(a_src), Q' = exp(-0.8 a_dst)
  Masked grid weight  G[j,i] = adjT[j,i] * max(E1_j Q'_i, E2_j).

Work split per (j-tile, head):
  - DVE-path tiles: T = tensor_scalar(Q'_bcast; mult E1, max E2)  (4x mode)
  - ACT-path tiles: T = relu(E1 * Q' - E2) on the Scalar engine (per-
    partition scale/bias APs); the missing separable E2_j branch is
    restored by ONE extra matmul per (tile, i-block) with the *shared*
    adjacency block as stationary and the E2-scaled xh of all 4 heads
    as moving operand:  num += (E2 xh)^T-block contribution.
  - both paths: G = tensor_tensor(T, adjT)  (2 heads per op, 2x mode)

Accumulation (flipped orientation — no epilogue transposes):
  For each (tile t, i-block ib of 128, head h):
      acc[ib][i, h, c|den] += G_block[j, i]^T @ xh1[j, (c|1)]
  i.e. the 128x128 grid block is the *stationary* operand (FWL-eligible
  fp16 128-col load) and xh1 streams 65 cols.  PSUM acc2[ib] is a full
  bank [128, 4, 128(pad)]; col 64 of each head slot is the softmax
  denominator.  Bias is pre-folded into xh1 (num+bias*den trick), so the
  epilogue is just reciprocal + per-partition tensor_scalar divide + DMA.

Q'_i is broadcast to all 128 partitions with a K=1 ones-stationary
matmul (PSUM bounce) instead of a DRAM roundtrip.

Sharding: core = b*2 + ihalf; each core owns 1024 destination rows of one
batch and reads that batch's full source side (adj slice pre-transposed,
self-loops added, fp16-cast on host; weights pre-folded with the per-head
attention vectors; x / W / projections run in fp16).
"""

import numpy as np

import concourse.bacc as bacc
import concourse.bass as bass
import concourse.tile as tile
from concourse import mybir
from concourse.bass_utils import run_bass_kernel_spmd
from concourse.masks import make_identity

B, N, F = 4, 2048, 128
H, C = 4, 64
HC = H * C
NEG_SLOPE = 0.2
import os
TBUFS = int(os.environ.get('TBUFS', 4))
GBUFS = int(os.environ.get('GBUFS', 5))
ABUFS = int(os.environ.get('ABUFS', 3))
ACTN = int(os.environ.get('ACTN', 8))   # of 16 j-tiles use the ACT path
N_CORES = 8
ID = N // 2          # dest rows per core
NT = N // 128        # 16 source tiles
NIB = ID // 128      # 8 dest 128-blocks
F32 = mybir.dt.float32
F16 = mybir.dt.float16

_NC_CACHE = {}


def act_tile_set(actn: int) -> set:
    """Spread actn ACT-path tiles over 1..NT-2 (t=0 / t=NT-1 stay DVE so
    start/stop accumulate flags land on grid matmuls)."""
    actn = max(0, min(actn, NT - 2))
    if actn == 0:
        return set()
    picks = np.linspace(1, NT - 2, actn)
    return set(int(round(p)) for p in picks)


def build_nc(reps: int = 1):
    nc = bacc.Bacc("TRN2", target_bir_lowering=False, debug=False, num_devices=1)

    d_xT = nc.dram_tensor("xT", [F, N], F16, kind="ExternalInput").ap()
    d_xTd = nc.dram_tensor("xTd", [F, ID], F16, kind="ExternalInput").ap()
    d_adjT = nc.dram_tensor("adjT", [NT, 128, ID], F16, kind="ExternalInput").ap()
    d_wcat = nc.dram_tensor("Wcat", [F, HC + 8], F16, kind="ExternalInput").ap()
    d_wadst = nc.dram_tensor("Wadst", [F, H], F16, kind="ExternalInput").ap()
    d_bias = nc.dram_tensor("biasv", [1, HC], F32, kind="ExternalInput").ap()
    d_out = nc.dram_tensor("out", [ID, HC], F32, kind="ExternalOutput").ap()

    EXP = mybir.ActivationFunctionType.Exp
    CPY = mybir.ActivationFunctionType.Copy
    RELU = mybir.ActivationFunctionType.Relu
    acts = act_tile_set(ACTN)

    with tile.TileContext(nc) as tc:
        with tc.tile_pool(name="const", bufs=1) as const:
            ones1 = const.tile([1, 128], F32)
            nc.vector.memset(ones1, 1.0)
            ones16 = const.tile([1, 128], F16)
            nc.vector.memset(ones16, 1.0)
            z128 = const.tile([1, 128], F16)
            nc.vector.memset(z128, 0.0)
            z512 = const.tile([1, 512], F16)
            nc.vector.memset(z512, 0.0)

            # preload the exp table set while input DMAs run
            scratch1 = const.tile([1, 4], F32)
            nc.scalar.activation(scratch1, ones1[0:1, 0:4], EXP)

            wcat = const.tile([F, HC + 8], F16)
            nc.sync.dma_start(out=wcat, in_=d_wcat)
            wadst = const.tile([F, H], F16)
            nc.sync.dma_start(out=wadst, in_=d_wadst)
            xTd = const.tile([F, ID], F16)
            nc.sync.dma_start(out=xTd, in_=d_xTd)
            xT = const.tile([F, N], F16)
            for c in range(2):
                nc.sync.dma_start(out=xT[:, c * 1024:(c + 1) * 1024],
                                  in_=d_xT[:, c * 1024:(c + 1) * 1024])
            bias_sb = const.tile([1, HC], F32)
            nc.sync.dma_start(out=bias_sb, in_=d_bias)

            # persistent per-core tensors
            xh1 = const.tile([128, NT, H, 65], F16)     # [xh+bias | 1] per (t,h)
            xh2b = const.tile([128, NT, H, 65], F16)    # E2-scaled xh1 (ACT tiles)
            expv = const.tile([128, NT, 8], F32)        # exp(.2 a_src) | exp(a_src)
            nexpv = const.tile([128, NT, 4], F32)       # -exp(a_src) (ACT bias)
            q_bc = const.tile([128, H, ID], F16)        # Q' broadcast per head
            qrow = const.tile([1, H, ID], F16)          # exp(-0.8 a_dst), row form
            bias16 = const.tile([128, HC], F16)

            # ---------------- phase A: projections ----------------
            with tc.tile_pool(name="psA", bufs=2, space="PSUM") as psA, \
                 tc.tile_pool(name="psD", bufs=2, space="PSUM") as psDp, \
                 tc.tile_pool(name="psQ", bufs=2, space="PSUM") as psQp, \
                 tc.tile_pool(name="psB", bufs=2, space="PSUM") as psBp:
                # ones column of every xh1 block (cols 0:64 written below)
                nc.gpsimd.memset(xh1[:, :, :, 64:65], 1.0)
                sc_a = nc.enter_named_scope("phA", False)
                # --- q_bc prefix first: it gates the grid loop.  a_dst
                # projection -> exp -> K=1 ones-stationary matmul broadcasts
                # each Q' row to all 128 partitions via PSUM (no DRAM bounce).
                for h in range(H):
                    for k in range(2):
                        psd = psDp.tile([1, 512], F32)
                        nc.tensor.matmul(psd, wadst[:, h:h + 1],
                                         xTd[:, k * 512:(k + 1) * 512],
                                         start=True, stop=True)
                        nc.scalar.activation(qrow[0:1, h, k * 512:(k + 1) * 512],
                                             psd, EXP)
                        psq = psQp.tile([128, 512], F32)
                        nc.tensor.matmul(psq, ones16,
                                         qrow[0:1, h, k * 512:(k + 1) * 512],
                                         start=True, stop=True)
                        nc.vector.tensor_copy(
                            q_bc[:, h, k * 512:(k + 1) * 512], psq)
                # bias broadcast to all partitions (folded into xh1 below)
                psb2 = psBp.tile([128, HC], F32, tag="psbias", bufs=1)
                nc.tensor.matmul(psb2, ones1, bias_sb, start=True, stop=True)
                nc.scalar.activation(bias16, psb2, CPY)
                # projection tiles; grid tile t can start once tile t is done
                for t in range(NT):
                    ps = psA.tile([128, HC + 8], F32)
                    nc.tensor.matmul(ps, xT[:, t * 128:(t + 1) * 128], wcat,
                                     start=True, stop=True)
                    # exp of the 8 pre-scaled projection cols
                    nc.scalar.activation(expv[:, t, :], ps[:, HC:HC + 8], EXP)
                    # raw xh into the 65-column head blocks
                    nc.scalar.activation(xh1[:, t, :, 0:64], ps[:, 0:HC], CPY)
                    # fold bias into xh: num+bias*den trick (den col untouched)
                    xh1v = bass.AP(
                        tensor=xh1.tensor,
                        offset=xh1.offset + (t * H) * 65,
                        ap=[xh1.ap[0], [65, H], [1, 64]])
                    b16v = bass.AP(
                        tensor=bias16.tensor, offset=bias16.offset,
                        ap=[bias16.ap[0], [64, H], [1, 64]])
                    nc.vector.tensor_tensor(out=xh1v, in0=xh1v, in1=b16v,
                                            op=mybir.AluOpType.add)
                    if t in acts:
                        # negated E2 for the ACT relu bias
                        nc.vector.tensor_scalar(
                            out=nexpv[:, t, :], in0=expv[:, t, 4:8],
                            scalar1=-1.0, scalar2=None,
                            op0=mybir.AluOpType.mult)
                        # E2-scaled stationary for the separable branch
                        for h in range(H):
                            nc.vector.tensor_scalar(
                                out=xh2b[:, t, h, :], in0=xh1[:, t, h, :],
                                scalar1=expv[:, t, 4 + h:5 + h], scalar2=None,
                                op0=mybir.AluOpType.mult)
                nc.leave_named_scope("phA", sc_a[0], False)

            # ---------------- phase B: grid + matmul accumulate ----------------
            with tc.tile_pool(name="acc", bufs=1, space="PSUM") as accp:
                acc = {}
                for ib in range(NIB):
                    acc[ib] = accp.tile([128, H, 128], F32, tag=f"acc{ib}",
                                        name=f"acc{ib}")

                sc_b = nc.enter_named_scope("phB", False)
                # one whole-bank zeroing matmul per acc bank: carries the only
                # start=True, so per-head accumulate groups sharing a bank
                # can't clear each other's has_written bits
                for ib in range(NIB):
                    accf = bass.AP(
                        tensor=acc[ib].tensor, offset=acc[ib].offset,
                        ap=[acc[ib].ap[0], [1, H * 128]])
                    nc.tensor.matmul(accf, z128, z512, start=True, stop=False)
                with tc.tile_pool(name="adj", bufs=ABUFS) as adjp, \
                     tc.tile_pool(name="grid", bufs=4) as gridp:
                    for rep in range(reps):
                        for t in range(NT):
                            adjt = adjp.tile([128, ID], F16)
                            nc.sync.dma_start(out=adjt, in_=d_adjT[t])
                            t_all = gridp.tile([128, H, ID], F16, tag="T", bufs=TBUFS)
                            for h in range(H):
                                if t in acts:
                                    # T = relu(E1_j * Q'_i - E2_j) on ACT
                                    nc.scalar.activation(
                                        t_all[:, h, :], q_bc[:, h, :], RELU,
                                        bias=nexpv[:, t, h:h + 1],
                                        scale=expv[:, t, h:h + 1])
                                else:
                                    # T = max(Q'_i * E1_j, E2_j) on DVE
                                    nc.vector.tensor_scalar(
                                        out=t_all[:, h, :], in0=q_bc[:, h, :],
                                        scalar1=expv[:, t, h:h + 1],
                                        scalar2=expv[:, t, 4 + h:5 + h],
                                        op0=mybir.AluOpType.mult,
                                        op1=mybir.AluOpType.max)
                            g = gridp.tile([128, H, ID], F16, tag="G", bufs=GBUFS)
                            adj_rep2 = bass.AP(
                                tensor=adjt.tensor, offset=adjt.offset,
                                ap=[adjt.ap[0], [0, 2]] + list(adjt.ap[1:]))
                            for hp in range(2):
                                nc.vector.tensor_tensor(
                                    out=g[:, hp * 2:(hp + 1) * 2, :],
                                    in0=t_all[:, hp * 2:(hp + 1) * 2, :],
                                    in1=adj_rep2,
                                    op=mybir.AluOpType.mult)
                            last = (rep == reps - 1 and t == NT - 1)
                            for ib in range(NIB):
                                for h in range(H):
                                    # acc[ib][i, h, :] += G_blk^T @ [xh|1]
                                    nc.tensor.matmul(
                                        acc[ib][:, h, 0:65],
                                        g[:, h, ib * 128:(ib + 1) * 128],
                                        xh1[:, t, h, :],
                                        start=False, stop=last)
                                if t in acts:
                                    # separable E2 branch: shared adj block
                                    # stationary, all 4 heads' E2-xh moving
                                    nc.tensor.matmul(
                                        acc[ib][:, :, 0:65],
                                        adjt[:, ib * 128:(ib + 1) * 128],
                                        xh2b[:, t, :, :],
                                        start=False, stop=False)

                nc.leave_named_scope("phB", sc_b[0], False)
                sc_c = nc.enter_named_scope("phC", False)
                # ------------- phase C: divide + out (no transposes) -------------
                with tc.tile_pool(name="ep_sm", bufs=4) as epsm, \
                     tc.tile_pool(name="outp", bufs=4) as outp:
                    for ib in range(NIB):
                        rec = epsm.tile([128, H, 1], F32)
                        nc.vector.reciprocal(rec, acc[ib][:, :, 64:65])
                        osb = outp.tile([128, HC], F32, tag="osb", name="osb")
                        for h in range(H):
                            eng = nc.vector if h % 2 == 0 else nc.scalar
                            if h % 2 == 0:
                                nc.vector.tensor_scalar(
                                    out=osb[:, h * 64:(h + 1) * 64],
                                    in0=acc[ib][:, h, 0:64],
                                    scalar1=rec[:, h, :], scalar2=None,
                                    op0=mybir.AluOpType.mult)
                            else:
                                nc.scalar.activation(
                                    osb[:, h * 64:(h + 1) * 64],
                                    acc[ib][:, h, 0:64], CPY,
                                    scale=rec[:, h, :])
                        blk = d_out[ib * 128:(ib + 1) * 128, :]
                        out_ap = bass.AP(
                            tensor=blk.tensor, offset=blk.offset,
                            ap=[[HC, 128], [1, HC]])
                        nc.sync.dma_start(out=out_ap, in_=osb)
                nc.leave_named_scope("phC", sc_c[0], False)

    nc.compile()
    return nc


def _get_nc(reps: int = 1):
    if reps not in _NC_CACHE:
        _NC_CACHE[reps] = build_nc(reps)
    return _NC_CACHE[reps]


def make_in_maps(x, adj, W, att_src, att_dst, bias):
    x = np.asarray(x, dtype=np.float32)
    adj = np.asarray(adj, dtype=np.float32)
    W = np.asarray(W, dtype=np.float32)
    att_src = np.asarray(att_src, dtype=np.float32)
    att_dst = np.asarray(att_dst, dtype=np.float32)
    bias = np.asarray(bias, dtype=np.float32)

    # weight prep: fold per-head attention dots into projection columns
    wa_src = np.stack([W[:, h * C:(h + 1) * C] @ att_src[h] for h in range(H)], 1)
    wa_dst = np.stack([W[:, h * C:(h + 1) * C] @ att_dst[h] for h in range(H)], 1)
    wcat = np.concatenate([W, 0.2 * wa_src, wa_src], axis=1)
    wcat = np.ascontiguousarray(wcat, dtype=np.float16)          # [F, 264]
    wadst = np.ascontiguousarray(-0.8 * wa_dst, dtype=np.float16)  # [F, 4]

    adjl = adj.copy()
    idx = np.arange(N)
    adjl[:, idx, idx] = 1.0

    in_maps = []
    for c in range(N_CORES):
        b, half = c // 2, c % 2
        xT = np.ascontiguousarray(x[b].T, dtype=np.float16)
        xTd = np.ascontiguousarray(x[b, half * ID:(half + 1) * ID, :].T,
                                   dtype=np.float16)
        adjT = np.ascontiguousarray(
            adjl[b].T[:, half * ID:(half + 1) * ID]).astype(np.float16)
        in_maps.append({
            "xT": xT,
            "xTd": xTd,
            "adjT": adjT.reshape(NT, 128, ID),
            "Wcat": wcat,
            "Wadst": wadst,
            "biasv": bias.reshape(1, HC),
        })
    return in_maps


def assemble(results):
    out = np.empty((B, N, HC), dtype=np.float32)
    for c in range(N_CORES):
        b, half = c // 2, c % 2
        out[b, half * ID:(half + 1) * ID, :] = results[c]["out"]
    return out


def kernel(x, adj, W, att_src, att_dst, bias):
    nc = _get_nc(1)
    in_maps = make_in_maps(x, adj, W, att_src, att_dst, bias)
    res = run_bass_kernel_spmd(nc, in_maps, list(range(N_CORES)))
    return assemble(res.results)
